# revision 5
# baseline (speedup 1.0000x reference)
"""L1-attention kernel for Trainium2 (8 NeuronCores).

attn[b, i, j, h] = -(1/sqrt(W)) * sum_w |q[b,j,h,w] - k[b,i,h,w]|

Strategy ("plan-M"):
  |a-b| = a + b - 2*min(a,b), so
  attn = s*Sq[j,h] + s*Sk[i,h] - 2s*sum_w min(q,k),  s = -1/8.

  Shard (batch x head-pair) across the 8 cores. Per core, layout
  q^T/k^T as [p=(head_sub,w)=128, seq=512] fp16 tiles. For each key i a
  single DVE tensor_scalar_min (4x perf mode) computes min(q^T, k_i);
  the PE reduces over the (head,w) partition axis with a per-key
  one-hot stationary that routes the two head-sums of key i into PSUM
  rows (2k, 2k+1), so 64 keys accumulate into one full [128,512] PSUM
  bank. Evacuation fuses the 0.25 scale and the rank-1 corrections
  (host-precomputed row/col sums) via ACT bias + one DVE add.
"""

import sys

sys.path.insert(0, "/opt/trn_rl_repo")

import numpy as np

BS, N_CTX, N_HEADS, WIDTH = 2, 512, 8, 64
N_CORES = 8
G = 8  # key groups per core
GK = 64  # keys per group
SCALE = -1.0 / 8.0

_CACHE = {}


def _build():
    if "nc" in _CACHE:
        return _CACHE["nc"]

    import concourse.bacc as bacc
    import concourse.mybir as mybir
    import concourse.tile as tile

    fp16 = mybir.dt.float16
    fp32 = mybir.dt.float32

    nc = bacc.Bacc(
        "TRN2",
        target_bir_lowering=False,
        debug=False,
        enable_asserts=True,
        num_devices=N_CORES,
    )

    qt_d = nc.dram_tensor("qt", [128, N_CTX], fp16, kind="ExternalInput")
    kt_d = nc.dram_tensor("kt", [128, N_CTX], fp32, kind="ExternalInput")
    sqb_d = nc.dram_tensor("sqb", [128, N_CTX], fp32, kind="ExternalInput")
    skb_d = nc.dram_tensor("skb", [128, G], fp32, kind="ExternalInput")
    out_d = nc.dram_tensor("out", [2, N_CTX, N_CTX], fp32, kind="ExternalOutput")

    # one-hot stationaries: stat[c, k, o] = 1 iff o == 2k + c//64
    stat_np = np.zeros((128, GK, 128), dtype=np.float16)
    c_idx = np.arange(128)
    for kk in range(GK):
        stat_np[c_idx, kk, 2 * kk + c_idx // 64] = 1.0
    stat_d = nc.inline_tensor(stat_np, name="stat")

    with tile.TileContext(nc) as tc:
        with (
            tc.tile_pool(name="const", bufs=1) as constp,
            tc.tile_pool(name="m", bufs=4) as mp,
            tc.tile_pool(name="ps", bufs=2, space="PSUM") as pp,
            tc.tile_pool(name="o", bufs=3) as outp,
        ):
            qt = constp.tile([128, N_CTX], fp16)
            kt = constp.tile([128, N_CTX], fp32)
            sqb = constp.tile([128, N_CTX], fp32)
            skb = constp.tile([128, G], fp32)
            stat = constp.tile([128, GK, 128], fp16)
            nc.sync.dma_start(qt[:], qt_d[:])
            nc.sync.dma_start(kt[:], kt_d[:])
            nc.sync.dma_start(sqb[:], sqb_d[:])
            nc.sync.dma_start(skb[:], skb_d[:])
            nc.sync.dma_start(stat[:], stat_d[:])

            for g in range(G):
                ps = pp.tile([128, N_CTX], fp32)
                for kk in range(GK):
                    i = g * GK + kk
                    m_t = mp.tile([128, N_CTX], fp16)
                    nc.vector.tensor_scalar_min(m_t[:], qt[:], kt[:, i : i + 1])
                    nc.tensor.matmul(
                        ps[:],
                        stat[:, kk, :],
                        m_t[:],
                        start=(kk == 0),
                        stop=(kk == GK - 1),
                    )
                t = outp.tile([128, N_CTX], fp32)
                nc.scalar.activation(
                    t[:],
                    ps[:],
                    mybir.ActivationFunctionType.Identity,
                    bias=skb[:, g : g + 1],
                    scale=0.25,
                )
                o = outp.tile([128, N_CTX], fp32)
                nc.vector.tensor_add(o[:], t[:], sqb[:])
                nc.sync.dma_start(
                    out_d[:, g * GK : (g + 1) * GK, :].rearrange("h i j -> i h j"),
                    o[:],
                )

    nc.compile()
    _CACHE["nc"] = nc
    return nc


def _core_inputs(q, k, c):
    b, hp = divmod(c, 4)
    heads = [2 * hp, 2 * hp + 1]
    qh = q[b][:, heads, :].astype(np.float16)  # [512, 2, 64]
    kh = k[b][:, heads, :].astype(np.float16)
    qt = np.ascontiguousarray(qh.transpose(1, 2, 0).reshape(128, N_CTX))
    kt = np.ascontiguousarray(kh.transpose(1, 2, 0).reshape(128, N_CTX)).astype(
        np.float32
    )
    sq = qh.astype(np.float32).sum(-1)  # [512, 2]
    sk = kh.astype(np.float32).sum(-1)  # [512, 2]
    sqb = np.empty((128, N_CTX), np.float32)
    sqb[0::2, :] = SCALE * sq[:, 0][None, :]
    sqb[1::2, :] = SCALE * sq[:, 1][None, :]
    skb = np.ascontiguousarray(
        (SCALE * sk).reshape(G, GK, 2).transpose(1, 2, 0).reshape(128, G)
    ).astype(np.float32)
    return {"qt": qt, "kt": kt, "sqb": sqb, "skb": skb}


def kernel(q, k, _trace=False):
    from concourse.bass_utils import run_bass_kernel_spmd

    q = np.asarray(q, dtype=np.float32)
    k = np.asarray(k, dtype=np.float32)
    nc = _build()
    in_maps = [_core_inputs(q, k, c) for c in range(N_CORES)]
    res = run_bass_kernel_spmd(nc, in_maps, core_ids=list(range(N_CORES)), trace=_trace)
    _CACHE["last_results"] = res
    attn = np.empty((BS, N_CTX, N_CTX, N_HEADS), np.float32)
    for c in range(N_CORES):
        b, hp = divmod(c, 4)
        o = res.results[c]["out"]
        attn[b, :, :, 2 * hp] = o[0]
        attn[b, :, :, 2 * hp + 1] = o[1]
    return attn
